# revision 5
# baseline (speedup 1.0000x reference)
"""DiscriminativeLoss TRN2 kernel v5 — transposed phase B, PE-heavy pipeline.

Per core: one batch element [N, 32] f32 + labels [N] i32 -> (pull_b, push_b).

Layouts (P=128 partitions, npc points/partition, point n = p*npc + c):
  oh_lj  [128, (l, c)] bf16   one-hot, l-major rows (32 TSP instrs, 4x mode)
  hi     [128, (c, d)] bf16   bf16 copy of emb (ACT), source for XBAR
  embT4  [128, m] bf16        transposed emb: row 32q+d, col m=ch*SQH+s*H+cc
                              (q = p//32 quarter, s = p%32, c = ch*H+cc)
  ohT4   [(q,l), m] bf16      transposed one-hot (DMA-replicated labels + TSP)

Phase A (seg sums):  1024 matmuls psum[32l, 32d] += oh_chunk^T @ hi_chunk
  counts via tensor_scalar accum (DVE) + ones matmul partition-reduce.
Phase B (transposed): per slab (ch, s) of 512 cols:
  diff_psum = (-meansT4) @ ohT4_slab + I128 @ embT4_slab   (2 accum matmuls)
  sqd = Square(diff_psum) [ACT] ; d2_psum += ones_s @ sqd  (accum over s)
  hinge: dist = sqrt(d2+eps), h = relu(dist - dv)          [ACT]
Phase C: segsum[l] via scalar_tensor_tensor(oh_row * h, accum_out) [DVE]
  + 2 accumulating matmuls for the cross-partition reduce.
Push tail: identical to baseline (f32, tiny).
"""

import sys

sys.path.insert(0, "/opt/trn_rl_repo")

import numpy as np
from contextlib import ExitStack

import concourse.bass as bass
import concourse.bacc as bacc
import concourse.mybir as mybir
import concourse.tile as tile

F32 = mybir.dt.float32
BF16 = mybir.dt.bfloat16
I32 = mybir.dt.int32
AX = mybir.AxisListType
OP = mybir.AluOpType
AF = mybir.ActivationFunctionType

D = 32
NL = 32          # instance labels 1..32 (label 0 ignored everywhere)
SQ = 32          # partitions per quarter
DELTA_V = 0.1
DELTA_D = 0.5


def emit(tc, emb_d, lab16_d, cnt_d, res_d, npc):
    nc = tc.nc
    ctx = tc.ctx
    P = 128
    H = npc // 2          # cols per ch-half (per s)
    SQH = SQ * H          # embT4 cols per ch
    NBLKH = npc // 64     # 32-chunk blocks per half

    emb_v = emb_d[:].rearrange("(p c) d -> p (c d)", p=P)
    lab_v = lab16_d[:].rearrange("(p c) -> p c", p=P)

    # ---------------- pools ----------------
    p_pers = ctx.enter_context(tc.tile_pool(name="p_pers", bufs=1))
    p_small = ctx.enter_context(tc.tile_pool(name="p_small", bufs=1))
    p_t2 = ctx.enter_context(tc.tile_pool(name="p_t2", bufs=3))   # epoch2 tiles
    ps_a = ctx.enter_context(tc.tile_pool(name="ps_a", bufs=1, space="PSUM"))
    ps_diff = ctx.enter_context(tc.tile_pool(name="ps_diff", bufs=2, space="PSUM"))
    ps_d2 = ctx.enter_context(tc.tile_pool(name="ps_d2", bufs=1, space="PSUM"))
    ps_misc = ctx.enter_context(tc.tile_pool(name="ps_misc", bufs=2, space="PSUM"))

    # ---------------- persistent ----------------
    oh_lj = p_pers.tile([P, NL * npc], BF16, tag="oh_lj")
    oh3 = oh_lj[:].rearrange("p (l c) -> p l c", c=npc)
    # hi_st = 32x32-block stream transpose of hi: hi_st[32q+d, 32c+s] =
    # bf16 emb of point (q,s,c), dim d.  Slab (ch, s) reads cols
    # {32*(ch*H+cc)+s} — a strided AP, so no shuffle DMA is needed.
    hi_st = p_pers.tile([P, npc * D], BF16, tag="hi_st")
    hst3 = hi_st[:].rearrange("p (c s) -> p c s", s=SQ)
    h_all = p_pers.tile([P, npc], BF16, tag="h_all")
    lab_b = p_pers.tile([P, npc], BF16, tag="lab_b")
    segsum = p_pers.tile([P, 2 * NL], F32, tag="segsum")
    junk = p_pers.tile([P, npc], BF16, tag="junk")

    # ---------------- small constants ----------------
    negmT4 = p_small.tile([P, P], BF16, tag="negmT4")
    nc.vector.memset(negmT4[:], 0.0)
    id128 = p_small.tile([P, P], BF16, tag="id128")
    ones128b = p_small.tile([P, P], BF16, tag="ones128b")
    nc.vector.memset(ones128b[:], 1.0)
    nc.gpsimd.affine_select(
        id128[:], ones128b[:], pattern=[[1, P]], base=0,
        channel_multiplier=-1, compare_op=OP.is_equal, fill=0.0,
    )
    # ones_s base: base[r, c] = 1 iff c == 32*(r//32) + 31; view offset 31-s
    ones_base = p_small.tile([P, P + SQ], BF16, tag="ones_base")
    nc.vector.memset(ones_base[:], 0.0)
    for q in range(4):
        nc.vector.memset(ones_base[SQ * q:SQ * (q + 1), SQ * q + 31:SQ * q + 32], 1.0)
    ones128f = p_small.tile([P, 1], F32, tag="ones128f")
    nc.vector.memset(ones128f[:], 1.0)
    # lvec: value (p % 32) + 1 per partition
    lvec_i = p_small.tile([P, 1], I32, tag="lvec_i")
    nc.gpsimd.iota(lvec_i[:], pattern=[[0, 1]], base=0, channel_multiplier=1)
    lvec_m = p_small.tile([P, 1], I32, tag="lvec_m")
    nc.vector.tensor_scalar(out=lvec_m[:], in0=lvec_i[:], scalar1=31,
                            scalar2=None, op0=OP.bitwise_and)
    lvec_f = p_small.tile([P, 1], F32, tag="lvec_f")
    nc.vector.tensor_copy(lvec_f[:], lvec_m[:])
    lvec = p_small.tile([P, 1], F32, tag="lvec")
    nc.vector.tensor_scalar(out=lvec[:], in0=lvec_f[:], scalar1=1.0,
                            scalar2=None, op0=OP.add)
    eps_b = p_small.tile([P, 1], F32, tag="eps_b")
    nc.vector.memset(eps_b[:], 1e-24)
    ndv_b = p_small.tile([P, 1], F32, tag="ndv_b")
    nc.vector.memset(ndv_b[:], -DELTA_V)
    # push-tail constants (f32, 32x32 world)
    ones32 = p_small.tile([32, 32], F32, tag="ones32")
    nc.vector.memset(ones32[:], 1.0)
    id32 = p_small.tile([32, 32], F32, tag="id32")
    nc.gpsimd.affine_select(
        id32[:], ones32[:], pattern=[[1, 32]], base=0,
        channel_multiplier=-1, compare_op=OP.is_equal, fill=0.0,
    )
    ones_k1 = p_small.tile([1, 32], F32, tag="ones_k1")
    nc.vector.memset(ones_k1[:], 1.0)
    ones32c = p_small.tile([32, 1], F32, tag="ones32c")
    nc.vector.memset(ones32c[:], 1.0)

    # ---------------- labels (bf16 from host) + counts (host bincount) ----
    nc.sync.dma_start(lab_b[:], lab_v)
    cnt_sb = p_small.tile([32, 1], F32, tag="cnt_sb")
    nc.sync.dma_start(cnt_sb[:], cnt_d[:].rearrange("(l o) -> l o", o=1))

    # one-hot rows (l-major): 32 TSP instrs, 4x DVE mode
    for l in range(NL):
        nc.vector.tensor_scalar(out=oh3[:, l, :], in0=lab_b[:],
                                scalar1=float(l + 1), scalar2=None,
                                op0=OP.is_equal)
    psum_a = ps_a.tile([32, D], F32, tag="psum_a")

    # ================= EPOCH 1: stream emb, phase A + block transpose =======
    # hib33: 32 chunks of [32 dims | 1 one] per buffer; the ones column rides
    # along in the phase-A matmul so counts land in psum_a[:, 32] for free.
    NBLK = npc // 32
    with tc.tile_pool(name="p_in", bufs=3) as p_in, \
         tc.tile_pool(name="p_hib", bufs=3) as p_hib:
        for b in range(NBLK):
            ta = p_in.tile([P, 1024], F32, tag="ta")
            nc.sync.dma_start(ta[:], emb_v[:, b * 1024:(b + 1) * 1024])
            hib = p_hib.tile([P, 1024], BF16, tag="hib")
            nc.scalar.copy(hib[:], ta[:])
            nc.vector.transpose(hi_st[:, b * 1024:(b + 1) * 1024], hib[:])
            for j in range(32):
                c = b * 32 + j
                nc.tensor.matmul(
                    psum_a[:], oh3[:, :, c],
                    hib[:, j * D:(j + 1) * D],
                    start=(c == 0), stop=(c == npc - 1),
                )

    # ================= means =================
    cnt_cl = p_small.tile([32, 1], F32, tag="cnt_cl")
    nc.vector.tensor_scalar(out=cnt_cl[:], in0=cnt_sb[:], scalar1=1.0,
                            scalar2=None, op0=OP.max)
    recip = p_small.tile([32, 1], F32, tag="recip")
    nc.vector.reciprocal(recip[:], cnt_cl[:])
    nrecip = p_small.tile([32, 1], F32, tag="nrecip")
    nc.vector.tensor_scalar(out=nrecip[:], in0=recip[:], scalar1=-1.0,
                            scalar2=None, op0=OP.mult)
    means_f = p_small.tile([32, 32], F32, tag="means_f")
    nc.vector.tensor_scalar(out=means_f[:], in0=psum_a[:], scalar1=recip[:],
                            scalar2=None, op0=OP.mult)
    negm_b = p_small.tile([32, 32], BF16, tag="negm_b")
    nc.scalar.activation(negm_b[:], psum_a[:], AF.Copy, scale=nrecip[:])
    # replicate into 4 diagonal blocks of negmT4 (partition-shifting DMAs)
    for q in range(4):
        nc.sync.dma_start(negmT4[SQ * q:SQ * (q + 1), SQ * q:SQ * q + 32],
                          negm_b[:])

    # ================= EPOCH 2: transposed phase B + phase C ================
    d2_bank = {}
    for ch in range(2):
        d2_bank[ch] = ps_d2.tile([P, H], F32, tag=f"d2_{ch}", name=f"d2_{ch}")
    labdr3 = lab16_d[:].rearrange("(q s c) -> q s c", q=4, s=SQ)
    for ch in range(2):
        for s in range(SQ):
            labT = p_t2.tile([P, H], BF16, tag="labT")
            nc.scalar.dma_start(
                labT[:],
                labdr3[:, s, ch * H:(ch + 1) * H]
                .unsqueeze(1).broadcast_to([4, SQ, H]),
            )
            ohT = p_t2.tile([P, H], BF16, tag="ohT")
            nc.gpsimd.tensor_scalar(out=ohT[:], in0=labT[:], scalar1=lvec[:],
                                    scalar2=None, op0=OP.is_equal)
            dpsum = ps_diff.tile([P, H], F32, tag="dpsum")
            nc.tensor.matmul(dpsum[:], negmT4[:], ohT[:], start=True, stop=False)
            nc.tensor.matmul(dpsum[:], id128[:], hst3[:, ch * H:(ch + 1) * H, s],
                             start=False, stop=True)
            sqd = p_t2.tile([P, H], BF16, tag="sqd")
            nc.scalar.activation(sqd[:], dpsum[:], AF.Square)
            nc.tensor.matmul(d2_bank[ch][:], ones_base[:, 31 - s:159 - s],
                             sqd[:], start=(s == 0), stop=(s == SQ - 1),
                             skip_group_check=True)
        dist = p_t2.tile([P, H], F32, tag="dist")
        nc.scalar.activation(dist[:], d2_bank[ch][:], AF.Sqrt, bias=eps_b[:])
        nc.scalar.activation(h_all[:, ch * H:(ch + 1) * H], dist[:],
                             AF.Relu, bias=ndv_b[:])
        for l in range(NL):
            eng = nc.vector
            jk = junk
            eng.scalar_tensor_tensor(
                out=jk[:, 0:H], in0=oh3[:, l, ch * H:(ch + 1) * H], scalar=1.0,
                in1=h_all[:, ch * H:(ch + 1) * H], op0=OP.mult, op1=OP.mult,
                accum_out=segsum[:, ch * NL + l:ch * NL + l + 1],
            )

    # phase C cross-partition reduce: two accumulating matmuls
    ps_seg = ps_misc.tile([32, 1], F32, tag="misc")
    nc.tensor.matmul(ps_seg[:], segsum[:, 0:NL], ones128f[:],
                     start=True, stop=False)
    nc.tensor.matmul(ps_seg[:], segsum[:, NL:2 * NL], ones128f[:],
                     start=False, stop=True)

    seg_mean = p_small.tile([32, 1], F32, tag="seg_mean")
    nc.vector.tensor_scalar(out=seg_mean[:], in0=ps_seg[:], scalar1=recip[:],
                            scalar2=None, op0=OP.mult)

    # ================= push tail (f32, tiny; same as baseline) =============
    mnsq = p_small.tile([32, 32], F32, tag="mnsq")
    nc.vector.tensor_tensor(out=mnsq[:], in0=means_f[:], in1=means_f[:], op=OP.mult)
    nrm2 = p_small.tile([32, 1], F32, tag="nrm2")
    nc.vector.reduce_sum(out=nrm2[:], in_=mnsq[:], axis=AX.X)
    nrm = p_small.tile([32, 1], F32, tag="nrm")
    nc.scalar.activation(nrm[:], nrm2[:], AF.Sqrt)
    nrm_cl = p_small.tile([32, 1], F32, tag="nrm_cl")
    nc.vector.tensor_scalar(out=nrm_cl[:], in0=nrm[:], scalar1=1e-12,
                            scalar2=None, op0=OP.max)
    rnrm = p_small.tile([32, 1], F32, tag="rnrm")
    nc.vector.reciprocal(rnrm[:], nrm_cl[:])
    mn = p_small.tile([32, 32], F32, tag="mn")
    nc.vector.tensor_scalar(out=mn[:], in0=means_f[:], scalar1=rnrm[:],
                            scalar2=None, op0=OP.mult)

    ps_mnt = ps_misc.tile([32, 32], F32, tag="misc")
    nc.tensor.transpose(ps_mnt[:], mn[:], id32[:])
    mnt = p_small.tile([32, 32], F32, tag="mnt")
    nc.vector.tensor_copy(mnt[:], ps_mnt[:])

    ps_g = ps_misc.tile([32, 32], F32, tag="misc")
    nc.tensor.matmul(ps_g[:], mnt[:], mnt[:], start=True, stop=True)

    mnsq2 = p_small.tile([32, 32], F32, tag="mnsq2")
    nc.vector.tensor_tensor(out=mnsq2[:], in0=mn[:], in1=mn[:], op=OP.mult)
    nsq = p_small.tile([32, 1], F32, tag="nsq")
    nc.vector.reduce_sum(out=nsq[:], in_=mnsq2[:], axis=AX.X)

    present = p_small.tile([32, 1], F32, tag="present")
    nc.vector.tensor_scalar(out=present[:], in0=cnt_sb[:], scalar1=0.0,
                            scalar2=None, op0=OP.is_gt)

    sq_a = p_small.tile([32, 32], F32, tag="sq_a")
    nc.vector.tensor_scalar(out=sq_a[:], in0=ps_g[:], scalar1=-2.0,
                            scalar2=nsq[:], op0=OP.mult, op1=OP.add)

    ps_row0 = ps_misc.tile([1, 32], F32, tag="misc")
    nc.tensor.matmul(ps_row0[:], nsq[:], id32[:], start=True, stop=True)
    nsqt_sb = p_small.tile([1, 32], F32, tag="nsqt_sb")
    nc.vector.tensor_copy(nsqt_sb[:], ps_row0[:])
    ps_row1 = ps_misc.tile([1, 32], F32, tag="misc")
    nc.tensor.matmul(ps_row1[:], present[:], id32[:], start=True, stop=True)
    prest_sb = p_small.tile([1, 32], F32, tag="prest_sb")
    nc.vector.tensor_copy(prest_sb[:], ps_row1[:])

    ps_bc = ps_misc.tile([32, 64], F32, tag="misc")
    nc.tensor.matmul(ps_bc[:, 0:32], ones_k1[:], nsqt_sb[:],
                     start=True, stop=True)
    nc.tensor.matmul(ps_bc[:, 32:64], ones_k1[:], prest_sb[:],
                     start=True, stop=True)
    nsq_j = p_small.tile([32, 32], F32, tag="nsq_j")
    nc.vector.tensor_copy(nsq_j[:], ps_bc[:, 0:32])
    pres_j = p_small.tile([32, 32], F32, tag="pres_j")
    nc.vector.tensor_copy(pres_j[:], ps_bc[:, 32:64])

    sq0 = p_small.tile([32, 32], F32, tag="sq0")
    nc.vector.tensor_tensor(out=sq0[:], in0=sq_a[:], in1=nsq_j[:], op=OP.add)
    sqc = p_small.tile([32, 32], F32, tag="sqc")
    nc.vector.tensor_scalar(out=sqc[:], in0=sq0[:], scalar1=0.0,
                            scalar2=None, op0=OP.max)
    dmat = p_small.tile([32, 32], F32, tag="dmat")
    nc.scalar.activation(dmat[:], sqc[:], AF.Sqrt, bias=eps_b[0:32, :])
    hp0 = p_small.tile([32, 32], F32, tag="hp0")
    nc.scalar.activation(hp0[:], dmat[:], AF.Relu, bias=ones32c[:], scale=-1.0)
    hp1 = p_small.tile([32, 32], F32, tag="hp1")
    nc.vector.tensor_scalar(out=hp1[:], in0=hp0[:], scalar1=present[:],
                            scalar2=None, op0=OP.mult)
    hp2 = p_small.tile([32, 32], F32, tag="hp2")
    nc.vector.tensor_tensor(out=hp2[:], in0=hp1[:], in1=pres_j[:], op=OP.mult)
    hp3 = p_small.tile([32, 32], F32, tag="hp3")
    nc.gpsimd.affine_select(hp3[:], hp2[:], pattern=[[1, 32]], base=0,
                            channel_multiplier=-1, compare_op=OP.is_gt, fill=0.0)
    pm1 = p_small.tile([32, 32], F32, tag="pm1")
    nc.vector.tensor_scalar(out=pm1[:], in0=pres_j[:], scalar1=present[:],
                            scalar2=None, op0=OP.mult)
    pm = p_small.tile([32, 32], F32, tag="pm")
    nc.gpsimd.affine_select(pm[:], pm1[:], pattern=[[1, 32]], base=0,
                            channel_multiplier=-1, compare_op=OP.is_gt, fill=0.0)
    hp_rs = p_small.tile([32, 1], F32, tag="hp_rs")
    nc.vector.reduce_sum(out=hp_rs[:], in_=hp3[:], axis=AX.X)
    pm_rs = p_small.tile([32, 1], F32, tag="pm_rs")
    nc.vector.reduce_sum(out=pm_rs[:], in_=pm[:], axis=AX.X)

    # ================= finals =================
    cat4 = p_small.tile([32, 4], F32, tag="cat4")
    nc.vector.tensor_copy(cat4[:, 0:1], seg_mean[:])
    nc.vector.tensor_copy(cat4[:, 1:2], present[:])
    nc.vector.tensor_copy(cat4[:, 2:3], hp_rs[:])
    nc.vector.tensor_copy(cat4[:, 3:4], pm_rs[:])
    ps_fin = ps_misc.tile([1, 4], F32, tag="misc")
    nc.tensor.matmul(ps_fin[:], ones32c[:], cat4[:], start=True, stop=True)
    sc = p_small.tile([1, 4], F32, tag="sc")
    nc.vector.tensor_copy(sc[:], ps_fin[:])

    res_sb = p_small.tile([1, 8], F32, tag="res_sb")
    nc.vector.memset(res_sb[:], 0.0)
    t1 = p_small.tile([1, 1], F32, tag="t1")
    nc.vector.tensor_scalar(out=t1[:], in0=sc[:, 1:2], scalar1=1e-6,
                            scalar2=None, op0=OP.add)
    r1 = p_small.tile([1, 1], F32, tag="r1")
    nc.vector.reciprocal(r1[:], t1[:])
    nc.vector.tensor_tensor(out=res_sb[:, 0:1], in0=sc[:, 0:1], in1=r1[:],
                            op=OP.mult)
    t2 = p_small.tile([1, 1], F32, tag="t2")
    nc.vector.tensor_scalar(out=t2[:], in0=sc[:, 3:4], scalar1=1e-6,
                            scalar2=None, op0=OP.add)
    r2 = p_small.tile([1, 1], F32, tag="r2")
    nc.vector.reciprocal(r2[:], t2[:])
    pb0 = p_small.tile([1, 1], F32, tag="pb0")
    nc.vector.tensor_tensor(out=pb0[:], in0=sc[:, 2:3], in1=r2[:], op=OP.mult)
    gate = p_small.tile([1, 1], F32, tag="gate")
    nc.vector.tensor_scalar(out=gate[:], in0=sc[:, 1:2], scalar1=1.0,
                            scalar2=None, op0=OP.is_gt)
    nc.vector.tensor_tensor(out=res_sb[:, 1:2], in0=pb0[:], in1=gate[:],
                            op=OP.mult)

    nc.sync.dma_start(res_d[:], res_sb[:])


def build_program(npc):
    n = npc * 128
    nc = bacc.Bacc("TRN2", target_bir_lowering=False, debug=False)
    emb_d = nc.dram_tensor("emb", [n, D], F32, kind="ExternalInput")
    lab16_d = nc.dram_tensor("lab16", [n], BF16, kind="ExternalInput")
    cnt_d = nc.dram_tensor("cnt", [32], F32, kind="ExternalInput")
    res_d = nc.dram_tensor("res", [1, 8], F32, kind="ExternalOutput")
    with tile.TileContext(nc) as tc:
        with ExitStack() as ctx:
            tc.ctx = ctx
            emit(tc, emb_d, lab16_d, cnt_d, res_d, npc)
    nc.compile()
    return nc


_NC_CACHE = {}


def _get_nc(npc):
    if npc not in _NC_CACHE:
        _NC_CACHE[npc] = build_program(npc)
    return _NC_CACHE[npc]


def kernel(embeddings, labels):
    embeddings = np.asarray(embeddings, dtype=np.float32)
    labels = np.asarray(labels, dtype=np.int32)
    bsz = embeddings.shape[0]
    npc = embeddings.shape[1] // 128
    nc = _get_nc(npc)

    from concourse.bass_utils import run_bass_kernel_spmd

    import ml_dtypes
    lab16 = labels.astype(np.float32).astype(ml_dtypes.bfloat16)
    counts = np.stack([
        np.bincount(labels[b], minlength=33)[1:33].astype(np.float32)
        for b in range(bsz)
    ])
    in_maps = [
        {"emb": np.ascontiguousarray(embeddings[b]),
         "lab16": np.ascontiguousarray(lab16[b]),
         "cnt": counts[b]}
        for b in range(bsz)
    ]
    out = run_bass_kernel_spmd(nc, in_maps, list(range(bsz)))
    res = np.stack([out.results[b]["res"][0] for b in range(bsz)])
    pull = res[:, 0].sum() / bsz
    push = res[:, 1].sum() / bsz
    return np.stack([pull + push, pull, push]).astype(np.float32)


# revision 6
# speedup vs baseline: 2.2808x; 2.2808x over previous
"""DiscriminativeLoss TRN2 kernel v5 — transposed phase B, PE-heavy pipeline.

Per core: one batch element [N, 32] f32 + labels [N] i32 -> (pull_b, push_b).

Layouts (P=128 partitions, npc points/partition, point n = p*npc + c):
  oh_lj  [128, (l, c)] bf16   one-hot, l-major rows (32 TSP instrs, 4x mode)
  hi     [128, (c, d)] bf16   bf16 copy of emb (ACT), source for XBAR
  embT4  [128, m] bf16        transposed emb: row 32q+d, col m=ch*SQH+s*H+cc
                              (q = p//32 quarter, s = p%32, c = ch*H+cc)
  ohT4   [(q,l), m] bf16      transposed one-hot (DMA-replicated labels + TSP)

Phase A (seg sums):  1024 matmuls psum[32l, 32d] += oh_chunk^T @ hi_chunk
  counts via tensor_scalar accum (DVE) + ones matmul partition-reduce.
Phase B (transposed): per slab (ch, s) of 512 cols:
  diff_psum = (-meansT4) @ ohT4_slab + I128 @ embT4_slab   (2 accum matmuls)
  sqd = Square(diff_psum) [ACT] ; d2_psum += ones_s @ sqd  (accum over s)
  hinge: dist = sqrt(d2+eps), h = relu(dist - dv)          [ACT]
Phase C: segsum[l] via scalar_tensor_tensor(oh_row * h, accum_out) [DVE]
  + 2 accumulating matmuls for the cross-partition reduce.
Push tail: identical to baseline (f32, tiny).
"""

import sys

sys.path.insert(0, "/opt/trn_rl_repo")

import numpy as np
from contextlib import ExitStack

import concourse.bass as bass
import concourse.bacc as bacc
import concourse.mybir as mybir
import concourse.tile as tile

F32 = mybir.dt.float32
BF16 = mybir.dt.bfloat16
I32 = mybir.dt.int32
AX = mybir.AxisListType
OP = mybir.AluOpType
AF = mybir.ActivationFunctionType

D = 32
NL = 32          # instance labels 1..32 (label 0 ignored everywhere)
SQ = 32          # partitions per quarter
DELTA_V = 0.1
DELTA_D = 0.5


def emit(tc, emb_d, lab16_d, cnt_d, res_d, npc):
    nc = tc.nc
    ctx = tc.ctx
    P = 128
    H = npc // 2          # cols per ch-half (per s)
    SQH = SQ * H          # embT4 cols per ch
    NBLKH = npc // 64     # 32-chunk blocks per half

    emb_v = emb_d[:].rearrange("(p c) d -> p (c d)", p=P)
    lab_v = lab16_d[:].rearrange("(p c) -> p c", p=P)

    # ---------------- pools ----------------
    p_pers = ctx.enter_context(tc.tile_pool(name="p_pers", bufs=1))
    p_small = ctx.enter_context(tc.tile_pool(name="p_small", bufs=1))
    p_t2 = ctx.enter_context(tc.tile_pool(name="p_t2", bufs=3))   # epoch2 tiles
    ps_a = ctx.enter_context(tc.tile_pool(name="ps_a", bufs=1, space="PSUM"))
    ps_diff = ctx.enter_context(tc.tile_pool(name="ps_diff", bufs=2, space="PSUM"))
    ps_d2 = ctx.enter_context(tc.tile_pool(name="ps_d2", bufs=1, space="PSUM"))
    ps_misc = ctx.enter_context(tc.tile_pool(name="ps_misc", bufs=2, space="PSUM"))

    # ---------------- persistent ----------------
    oh_lj = p_pers.tile([P, NL * npc], BF16, tag="oh_lj")
    oh3 = oh_lj[:].rearrange("p (l c) -> p l c", c=npc)
    # hi_st = 32x32-block stream transpose of hi: hi_st[32q+d, 32c+s] =
    # bf16 emb of point (q,s,c), dim d.  Slab (ch, s) reads cols
    # {32*(ch*H+cc)+s} — a strided AP, so no shuffle DMA is needed.
    hi_st = p_pers.tile([P, npc * D], BF16, tag="hi_st")
    hst3 = hi_st[:].rearrange("p (c s) -> p c s", s=SQ)
    h_all = p_pers.tile([P, npc], BF16, tag="h_all")
    lab_b = p_pers.tile([P, npc], BF16, tag="lab_b")
    segsum = p_pers.tile([P, 2 * NL], F32, tag="segsum")
    junk = p_pers.tile([P, npc], BF16, tag="junk")

    # ---------------- small constants ----------------
    negmT4 = p_small.tile([P, P], BF16, tag="negmT4")
    nc.vector.memset(negmT4[:], 0.0)
    id128 = p_small.tile([P, P], BF16, tag="id128")
    ones128b = p_small.tile([P, P], BF16, tag="ones128b")
    nc.vector.memset(ones128b[:], 1.0)
    nc.gpsimd.affine_select(
        id128[:], ones128b[:], pattern=[[1, P]], base=0,
        channel_multiplier=-1, compare_op=OP.is_equal, fill=0.0,
    )
    # ones_s base: base[r, c] = 1 iff c == 32*(r//32) + 31; view offset 31-s
    ones_base = p_small.tile([P, P + SQ], BF16, tag="ones_base")
    nc.vector.memset(ones_base[:], 0.0)
    for q in range(4):
        nc.vector.memset(ones_base[SQ * q:SQ * (q + 1), SQ * q + 31:SQ * q + 32], 1.0)
    ones128f = p_small.tile([P, 1], F32, tag="ones128f")
    nc.vector.memset(ones128f[:], 1.0)
    # lvec: value (p % 32) + 1 per partition
    lvec_i = p_small.tile([P, 1], I32, tag="lvec_i")
    nc.gpsimd.iota(lvec_i[:], pattern=[[0, 1]], base=0, channel_multiplier=1)
    lvec_m = p_small.tile([P, 1], I32, tag="lvec_m")
    nc.vector.tensor_scalar(out=lvec_m[:], in0=lvec_i[:], scalar1=31,
                            scalar2=None, op0=OP.bitwise_and)
    lvec_f = p_small.tile([P, 1], F32, tag="lvec_f")
    nc.vector.tensor_copy(lvec_f[:], lvec_m[:])
    lvec = p_small.tile([P, 1], F32, tag="lvec")
    nc.vector.tensor_scalar(out=lvec[:], in0=lvec_f[:], scalar1=1.0,
                            scalar2=None, op0=OP.add)
    eps_b = p_small.tile([P, 1], F32, tag="eps_b")
    nc.vector.memset(eps_b[:], 1e-24)
    ndv_b = p_small.tile([P, 1], F32, tag="ndv_b")
    nc.vector.memset(ndv_b[:], -DELTA_V)
    # push-tail constants (f32, 32x32 world)
    ones32 = p_small.tile([32, 32], F32, tag="ones32")
    nc.vector.memset(ones32[:], 1.0)
    id32 = p_small.tile([32, 32], F32, tag="id32")
    nc.gpsimd.affine_select(
        id32[:], ones32[:], pattern=[[1, 32]], base=0,
        channel_multiplier=-1, compare_op=OP.is_equal, fill=0.0,
    )
    ones_k1 = p_small.tile([1, 32], F32, tag="ones_k1")
    nc.vector.memset(ones_k1[:], 1.0)
    ones32c = p_small.tile([32, 1], F32, tag="ones32c")
    nc.vector.memset(ones32c[:], 1.0)

    # ---------------- labels (bf16 from host) + counts (host bincount) ----
    nc.sync.dma_start(lab_b[:], lab_v)
    cnt_sb = p_small.tile([32, 1], F32, tag="cnt_sb")
    nc.sync.dma_start(cnt_sb[:], cnt_d[:].rearrange("(l o) -> l o", o=1))

    # one-hot rows (l-major): 32 TSP instrs, 4x DVE mode
    for l in range(NL):
        nc.vector.tensor_scalar(out=oh3[:, l, :], in0=lab_b[:],
                                scalar1=float(l + 1), scalar2=None,
                                op0=OP.is_equal)
    psum_a = ps_a.tile([32, D], F32, tag="psum_a")

    # ================= EPOCH 1: stream emb, phase A + block transpose =======
    # hib33: 32 chunks of [32 dims | 1 one] per buffer; the ones column rides
    # along in the phase-A matmul so counts land in psum_a[:, 32] for free.
    NBLK = npc // 32
    with tc.tile_pool(name="p_in", bufs=3) as p_in, \
         tc.tile_pool(name="p_hib", bufs=3) as p_hib:
        for b in range(NBLK):
            ta = p_in.tile([P, 1024], F32, tag="ta")
            nc.sync.dma_start(ta[:], emb_v[:, b * 1024:(b + 1) * 1024])
            hib = p_hib.tile([P, 1024], BF16, tag="hib")
            nc.scalar.copy(hib[:], ta[:])
            nc.vector.transpose(hi_st[:, b * 1024:(b + 1) * 1024], hib[:])
            for j in range(32):
                c = b * 32 + j
                nc.tensor.matmul(
                    psum_a[:], oh3[:, :, c],
                    hib[:, j * D:(j + 1) * D],
                    start=(c == 0), stop=(c == npc - 1),
                )

    # ================= means =================
    cnt_cl = p_small.tile([32, 1], F32, tag="cnt_cl")
    nc.vector.tensor_scalar(out=cnt_cl[:], in0=cnt_sb[:], scalar1=1.0,
                            scalar2=None, op0=OP.max)
    recip = p_small.tile([32, 1], F32, tag="recip")
    nc.vector.reciprocal(recip[:], cnt_cl[:])
    nrecip = p_small.tile([32, 1], F32, tag="nrecip")
    nc.vector.tensor_scalar(out=nrecip[:], in0=recip[:], scalar1=-1.0,
                            scalar2=None, op0=OP.mult)
    means_f = p_small.tile([32, 32], F32, tag="means_f")
    nc.vector.tensor_scalar(out=means_f[:], in0=psum_a[:], scalar1=recip[:],
                            scalar2=None, op0=OP.mult)
    negm_b = p_small.tile([32, 32], BF16, tag="negm_b")
    nc.scalar.activation(negm_b[:], psum_a[:], AF.Copy, scale=nrecip[:])
    # replicate into 4 diagonal blocks of negmT4 (partition-shifting DMAs)
    for q in range(4):
        nc.sync.dma_start(negmT4[SQ * q:SQ * (q + 1), SQ * q:SQ * q + 32],
                          negm_b[:])

    # ================= EPOCH 2: transposed phase B + phase C ================
    d2_bank = {}
    for ch in range(2):
        d2_bank[ch] = ps_d2.tile([P, H], F32, tag=f"d2_{ch}", name=f"d2_{ch}")
    labdr3 = lab16_d[:].rearrange("(q s c) -> q s c", q=4, s=SQ)
    for ch in range(2):
        for s in range(SQ):
            labT = p_t2.tile([P, H], BF16, tag="labT")
            nc.scalar.dma_start(
                labT[:],
                labdr3[:, s, ch * H:(ch + 1) * H]
                .unsqueeze(1).broadcast_to([4, SQ, H]),
            )
            ohT = p_t2.tile([P, H], BF16, tag="ohT")
            nc.vector.tensor_scalar(out=ohT[:], in0=labT[:], scalar1=lvec[:],
                                    scalar2=None, op0=OP.is_equal)
            dpsum = ps_diff.tile([P, H], F32, tag="dpsum")
            nc.tensor.matmul(dpsum[:], negmT4[:], ohT[:], start=True, stop=False)
            nc.tensor.matmul(dpsum[:], id128[:], hst3[:, ch * H:(ch + 1) * H, s],
                             start=False, stop=True)
            sqd = p_t2.tile([P, H], BF16, tag="sqd")
            nc.scalar.activation(sqd[:], dpsum[:], AF.Square)
            nc.tensor.matmul(d2_bank[ch][:], ones_base[:, 31 - s:159 - s],
                             sqd[:], start=(s == 0), stop=(s == SQ - 1),
                             skip_group_check=True)
        dist = p_t2.tile([P, H], F32, tag="dist")
        nc.scalar.activation(dist[:], d2_bank[ch][:], AF.Sqrt, bias=eps_b[:])
        nc.scalar.activation(h_all[:, ch * H:(ch + 1) * H], dist[:],
                             AF.Relu, bias=ndv_b[:])
        for l in range(NL):
            prod = p_t2.tile([P, H], BF16, tag="prod")
            nc.vector.tensor_tensor(
                out=prod[:], in0=oh3[:, l, ch * H:(ch + 1) * H],
                in1=h_all[:, ch * H:(ch + 1) * H], op=OP.mult)
            col = segsum[:, ch * NL + l:ch * NL + l + 1]
            if l < 16:
                nc.scalar.activation(junk[:, 0:H], prod[:], AF.Copy,
                                     accum_out=col)
            else:
                nc.vector.reduce_sum(out=col, in_=prod[:], axis=AX.X)

    # phase C cross-partition reduce: two accumulating matmuls
    ps_seg = ps_misc.tile([32, 1], F32, tag="misc")
    nc.tensor.matmul(ps_seg[:], segsum[:, 0:NL], ones128f[:],
                     start=True, stop=False)
    nc.tensor.matmul(ps_seg[:], segsum[:, NL:2 * NL], ones128f[:],
                     start=False, stop=True)

    seg_mean = p_small.tile([32, 1], F32, tag="seg_mean")
    nc.vector.tensor_scalar(out=seg_mean[:], in0=ps_seg[:], scalar1=recip[:],
                            scalar2=None, op0=OP.mult)

    # ================= push tail (f32, tiny; same as baseline) =============
    mnsq = p_small.tile([32, 32], F32, tag="mnsq")
    nc.vector.tensor_tensor(out=mnsq[:], in0=means_f[:], in1=means_f[:], op=OP.mult)
    nrm2 = p_small.tile([32, 1], F32, tag="nrm2")
    nc.vector.reduce_sum(out=nrm2[:], in_=mnsq[:], axis=AX.X)
    nrm = p_small.tile([32, 1], F32, tag="nrm")
    nc.scalar.activation(nrm[:], nrm2[:], AF.Sqrt)
    nrm_cl = p_small.tile([32, 1], F32, tag="nrm_cl")
    nc.vector.tensor_scalar(out=nrm_cl[:], in0=nrm[:], scalar1=1e-12,
                            scalar2=None, op0=OP.max)
    rnrm = p_small.tile([32, 1], F32, tag="rnrm")
    nc.vector.reciprocal(rnrm[:], nrm_cl[:])
    mn = p_small.tile([32, 32], F32, tag="mn")
    nc.vector.tensor_scalar(out=mn[:], in0=means_f[:], scalar1=rnrm[:],
                            scalar2=None, op0=OP.mult)

    ps_mnt = ps_misc.tile([32, 32], F32, tag="misc")
    nc.tensor.transpose(ps_mnt[:], mn[:], id32[:])
    mnt = p_small.tile([32, 32], F32, tag="mnt")
    nc.vector.tensor_copy(mnt[:], ps_mnt[:])

    ps_g = ps_misc.tile([32, 32], F32, tag="misc")
    nc.tensor.matmul(ps_g[:], mnt[:], mnt[:], start=True, stop=True)

    mnsq2 = p_small.tile([32, 32], F32, tag="mnsq2")
    nc.vector.tensor_tensor(out=mnsq2[:], in0=mn[:], in1=mn[:], op=OP.mult)
    nsq = p_small.tile([32, 1], F32, tag="nsq")
    nc.vector.reduce_sum(out=nsq[:], in_=mnsq2[:], axis=AX.X)

    present = p_small.tile([32, 1], F32, tag="present")
    nc.vector.tensor_scalar(out=present[:], in0=cnt_sb[:], scalar1=0.0,
                            scalar2=None, op0=OP.is_gt)

    sq_a = p_small.tile([32, 32], F32, tag="sq_a")
    nc.vector.tensor_scalar(out=sq_a[:], in0=ps_g[:], scalar1=-2.0,
                            scalar2=nsq[:], op0=OP.mult, op1=OP.add)

    ps_row0 = ps_misc.tile([1, 32], F32, tag="misc")
    nc.tensor.matmul(ps_row0[:], nsq[:], id32[:], start=True, stop=True)
    nsqt_sb = p_small.tile([1, 32], F32, tag="nsqt_sb")
    nc.vector.tensor_copy(nsqt_sb[:], ps_row0[:])
    ps_row1 = ps_misc.tile([1, 32], F32, tag="misc")
    nc.tensor.matmul(ps_row1[:], present[:], id32[:], start=True, stop=True)
    prest_sb = p_small.tile([1, 32], F32, tag="prest_sb")
    nc.vector.tensor_copy(prest_sb[:], ps_row1[:])

    ps_bc = ps_misc.tile([32, 64], F32, tag="misc")
    nc.tensor.matmul(ps_bc[:, 0:32], ones_k1[:], nsqt_sb[:],
                     start=True, stop=True)
    nc.tensor.matmul(ps_bc[:, 32:64], ones_k1[:], prest_sb[:],
                     start=True, stop=True)
    nsq_j = p_small.tile([32, 32], F32, tag="nsq_j")
    nc.vector.tensor_copy(nsq_j[:], ps_bc[:, 0:32])
    pres_j = p_small.tile([32, 32], F32, tag="pres_j")
    nc.vector.tensor_copy(pres_j[:], ps_bc[:, 32:64])

    sq0 = p_small.tile([32, 32], F32, tag="sq0")
    nc.vector.tensor_tensor(out=sq0[:], in0=sq_a[:], in1=nsq_j[:], op=OP.add)
    sqc = p_small.tile([32, 32], F32, tag="sqc")
    nc.vector.tensor_scalar(out=sqc[:], in0=sq0[:], scalar1=0.0,
                            scalar2=None, op0=OP.max)
    dmat = p_small.tile([32, 32], F32, tag="dmat")
    nc.scalar.activation(dmat[:], sqc[:], AF.Sqrt, bias=eps_b[0:32, :])
    hp0 = p_small.tile([32, 32], F32, tag="hp0")
    nc.scalar.activation(hp0[:], dmat[:], AF.Relu, bias=ones32c[:], scale=-1.0)
    hp1 = p_small.tile([32, 32], F32, tag="hp1")
    nc.vector.tensor_scalar(out=hp1[:], in0=hp0[:], scalar1=present[:],
                            scalar2=None, op0=OP.mult)
    hp2 = p_small.tile([32, 32], F32, tag="hp2")
    nc.vector.tensor_tensor(out=hp2[:], in0=hp1[:], in1=pres_j[:], op=OP.mult)
    hp3 = p_small.tile([32, 32], F32, tag="hp3")
    nc.gpsimd.affine_select(hp3[:], hp2[:], pattern=[[1, 32]], base=0,
                            channel_multiplier=-1, compare_op=OP.is_gt, fill=0.0)
    pm1 = p_small.tile([32, 32], F32, tag="pm1")
    nc.vector.tensor_scalar(out=pm1[:], in0=pres_j[:], scalar1=present[:],
                            scalar2=None, op0=OP.mult)
    pm = p_small.tile([32, 32], F32, tag="pm")
    nc.gpsimd.affine_select(pm[:], pm1[:], pattern=[[1, 32]], base=0,
                            channel_multiplier=-1, compare_op=OP.is_gt, fill=0.0)
    hp_rs = p_small.tile([32, 1], F32, tag="hp_rs")
    nc.vector.reduce_sum(out=hp_rs[:], in_=hp3[:], axis=AX.X)
    pm_rs = p_small.tile([32, 1], F32, tag="pm_rs")
    nc.vector.reduce_sum(out=pm_rs[:], in_=pm[:], axis=AX.X)

    # ================= finals =================
    cat4 = p_small.tile([32, 4], F32, tag="cat4")
    nc.vector.tensor_copy(cat4[:, 0:1], seg_mean[:])
    nc.vector.tensor_copy(cat4[:, 1:2], present[:])
    nc.vector.tensor_copy(cat4[:, 2:3], hp_rs[:])
    nc.vector.tensor_copy(cat4[:, 3:4], pm_rs[:])
    ps_fin = ps_misc.tile([1, 4], F32, tag="misc")
    nc.tensor.matmul(ps_fin[:], ones32c[:], cat4[:], start=True, stop=True)
    sc = p_small.tile([1, 4], F32, tag="sc")
    nc.vector.tensor_copy(sc[:], ps_fin[:])

    res_sb = p_small.tile([1, 8], F32, tag="res_sb")
    nc.vector.memset(res_sb[:], 0.0)
    t1 = p_small.tile([1, 1], F32, tag="t1")
    nc.vector.tensor_scalar(out=t1[:], in0=sc[:, 1:2], scalar1=1e-6,
                            scalar2=None, op0=OP.add)
    r1 = p_small.tile([1, 1], F32, tag="r1")
    nc.vector.reciprocal(r1[:], t1[:])
    nc.vector.tensor_tensor(out=res_sb[:, 0:1], in0=sc[:, 0:1], in1=r1[:],
                            op=OP.mult)
    t2 = p_small.tile([1, 1], F32, tag="t2")
    nc.vector.tensor_scalar(out=t2[:], in0=sc[:, 3:4], scalar1=1e-6,
                            scalar2=None, op0=OP.add)
    r2 = p_small.tile([1, 1], F32, tag="r2")
    nc.vector.reciprocal(r2[:], t2[:])
    pb0 = p_small.tile([1, 1], F32, tag="pb0")
    nc.vector.tensor_tensor(out=pb0[:], in0=sc[:, 2:3], in1=r2[:], op=OP.mult)
    gate = p_small.tile([1, 1], F32, tag="gate")
    nc.vector.tensor_scalar(out=gate[:], in0=sc[:, 1:2], scalar1=1.0,
                            scalar2=None, op0=OP.is_gt)
    nc.vector.tensor_tensor(out=res_sb[:, 1:2], in0=pb0[:], in1=gate[:],
                            op=OP.mult)

    nc.sync.dma_start(res_d[:], res_sb[:])


def build_program(npc):
    n = npc * 128
    nc = bacc.Bacc("TRN2", target_bir_lowering=False, debug=False)
    emb_d = nc.dram_tensor("emb", [n, D], F32, kind="ExternalInput")
    lab16_d = nc.dram_tensor("lab16", [n], BF16, kind="ExternalInput")
    cnt_d = nc.dram_tensor("cnt", [32], F32, kind="ExternalInput")
    res_d = nc.dram_tensor("res", [1, 8], F32, kind="ExternalOutput")
    with tile.TileContext(nc) as tc:
        with ExitStack() as ctx:
            tc.ctx = ctx
            emit(tc, emb_d, lab16_d, cnt_d, res_d, npc)
    nc.compile()
    return nc


_NC_CACHE = {}


def _get_nc(npc):
    if npc not in _NC_CACHE:
        _NC_CACHE[npc] = build_program(npc)
    return _NC_CACHE[npc]


def kernel(embeddings, labels):
    embeddings = np.asarray(embeddings, dtype=np.float32)
    labels = np.asarray(labels, dtype=np.int32)
    bsz = embeddings.shape[0]
    npc = embeddings.shape[1] // 128
    nc = _get_nc(npc)

    from concourse.bass_utils import run_bass_kernel_spmd

    import ml_dtypes
    lab16 = labels.astype(np.float32).astype(ml_dtypes.bfloat16)
    counts = np.stack([
        np.bincount(labels[b], minlength=33)[1:33].astype(np.float32)
        for b in range(bsz)
    ])
    in_maps = [
        {"emb": np.ascontiguousarray(embeddings[b]),
         "lab16": np.ascontiguousarray(lab16[b]),
         "cnt": counts[b]}
        for b in range(bsz)
    ]
    out = run_bass_kernel_spmd(nc, in_maps, list(range(bsz)))
    res = np.stack([out.results[b]["res"][0] for b in range(bsz)])
    pull = res[:, 0].sum() / bsz
    push = res[:, 1].sum() / bsz
    return np.stack([pull + push, pull, push]).astype(np.float32)


# revision 7
# speedup vs baseline: 2.7540x; 1.2075x over previous
"""DiscriminativeLoss TRN2 kernel v5 — transposed phase B, PE-heavy pipeline.

Per core: one batch element [N, 32] f32 + labels [N] i32 -> (pull_b, push_b).

Layouts (P=128 partitions, npc points/partition, point n = p*npc + c):
  oh_lj  [128, (l, c)] bf16   one-hot, l-major rows (32 TSP instrs, 4x mode)
  hi     [128, (c, d)] bf16   bf16 copy of emb (ACT), source for XBAR
  embT4  [128, m] bf16        transposed emb: row 32q+d, col m=ch*SQH+s*H+cc
                              (q = p//32 quarter, s = p%32, c = ch*H+cc)
  ohT4   [(q,l), m] bf16      transposed one-hot (DMA-replicated labels + TSP)

Phase A (seg sums):  1024 matmuls psum[32l, 32d] += oh_chunk^T @ hi_chunk
  counts via tensor_scalar accum (DVE) + ones matmul partition-reduce.
Phase B (transposed): per slab (ch, s) of 512 cols:
  diff_psum = (-meansT4) @ ohT4_slab + I128 @ embT4_slab   (2 accum matmuls)
  sqd = Square(diff_psum) [ACT] ; d2_psum += ones_s @ sqd  (accum over s)
  hinge: dist = sqrt(d2+eps), h = relu(dist - dv)          [ACT]
Phase C: segsum[l] via scalar_tensor_tensor(oh_row * h, accum_out) [DVE]
  + 2 accumulating matmuls for the cross-partition reduce.
Push tail: identical to baseline (f32, tiny).
"""

import sys

sys.path.insert(0, "/opt/trn_rl_repo")

import numpy as np
from contextlib import ExitStack

import concourse.bass as bass
import concourse.bacc as bacc
import concourse.mybir as mybir
import concourse.tile as tile

F32 = mybir.dt.float32
BF16 = mybir.dt.bfloat16
I32 = mybir.dt.int32
AX = mybir.AxisListType
OP = mybir.AluOpType
AF = mybir.ActivationFunctionType

D = 32
NL = 32          # instance labels 1..32 (label 0 ignored everywhere)
SQ = 32          # partitions per quarter
DELTA_V = 0.1
DELTA_D = 0.5


def emit(tc, emb_d, lab16_d, cnt_d, res_d, npc):
    nc = tc.nc
    ctx = tc.ctx
    P = 128
    H = npc // 2          # cols per ch-half (per s)
    SQH = SQ * H          # embT4 cols per ch
    NBLKH = npc // 64     # 32-chunk blocks per half

    emb_v = emb_d[:].rearrange("(p c) d -> p (c d)", p=P)
    lab_v = lab16_d[:].rearrange("(p c) -> p c", p=P)

    # ---------------- pools ----------------
    p_pers = ctx.enter_context(tc.tile_pool(name="p_pers", bufs=1))
    p_small = ctx.enter_context(tc.tile_pool(name="p_small", bufs=1))
    p_t2 = ctx.enter_context(tc.tile_pool(name="p_t2", bufs=4))   # epoch2 tiles
    ps_a = ctx.enter_context(tc.tile_pool(name="ps_a", bufs=1, space="PSUM"))
    ps_diff = ctx.enter_context(tc.tile_pool(name="ps_diff", bufs=3, space="PSUM"))
    ps_d2 = ctx.enter_context(tc.tile_pool(name="ps_d2", bufs=1, space="PSUM"))
    ps_misc = ctx.enter_context(tc.tile_pool(name="ps_misc", bufs=2, space="PSUM"))

    # ---------------- persistent ----------------
    oh_lj = p_pers.tile([P, NL * npc], BF16, tag="oh_lj")
    oh3 = oh_lj[:].rearrange("p (l c) -> p l c", c=npc)
    # hi_st = 32x32-block stream transpose of hi: hi_st[32q+d, 32c+s] =
    # bf16 emb of point (q,s,c), dim d.  Slab (ch, s) reads cols
    # {32*(ch*H+cc)+s} — a strided AP, so no shuffle DMA is needed.
    hi_st = p_pers.tile([P, npc * D], BF16, tag="hi_st")
    hst3 = hi_st[:].rearrange("p (c s) -> p c s", s=SQ)
    h_all = p_pers.tile([P, npc], BF16, tag="h_all")
    lab_b = p_pers.tile([P, npc], BF16, tag="lab_b")
    segsum = p_pers.tile([P, 2 * NL], F32, tag="segsum")
    junk = p_pers.tile([P, npc], BF16, tag="junk")

    # ---------------- small constants ----------------
    negmT4 = p_small.tile([P, P], BF16, tag="negmT4")
    nc.vector.memset(negmT4[:], 0.0)
    id128 = p_small.tile([P, P], BF16, tag="id128")
    ones128b = p_small.tile([P, P], BF16, tag="ones128b")
    nc.vector.memset(ones128b[:], 1.0)
    nc.gpsimd.affine_select(
        id128[:], ones128b[:], pattern=[[1, P]], base=0,
        channel_multiplier=-1, compare_op=OP.is_equal, fill=0.0,
    )
    # ones_s base: base[r, c] = 1 iff c == 32*(r//32) + 31; view offset 31-s
    ones_base = p_small.tile([P, P + SQ], BF16, tag="ones_base")
    nc.vector.memset(ones_base[:], 0.0)
    for q in range(4):
        nc.vector.memset(ones_base[SQ * q:SQ * (q + 1), SQ * q + 31:SQ * q + 32], 1.0)
    ones128f = p_small.tile([P, 1], F32, tag="ones128f")
    nc.vector.memset(ones128f[:], 1.0)
    # lvec: value (p % 32) + 1 per partition
    lvec_i = p_small.tile([P, 1], I32, tag="lvec_i")
    nc.gpsimd.iota(lvec_i[:], pattern=[[0, 1]], base=0, channel_multiplier=1)
    lvec_m = p_small.tile([P, 1], I32, tag="lvec_m")
    nc.vector.tensor_scalar(out=lvec_m[:], in0=lvec_i[:], scalar1=31,
                            scalar2=None, op0=OP.bitwise_and)
    lvec_f = p_small.tile([P, 1], F32, tag="lvec_f")
    nc.vector.tensor_copy(lvec_f[:], lvec_m[:])
    lvec = p_small.tile([P, 1], F32, tag="lvec")
    nc.vector.tensor_scalar(out=lvec[:], in0=lvec_f[:], scalar1=1.0,
                            scalar2=None, op0=OP.add)
    eps_b = p_small.tile([P, 1], F32, tag="eps_b")
    nc.vector.memset(eps_b[:], 1e-24)
    ndv_b = p_small.tile([P, 1], F32, tag="ndv_b")
    nc.vector.memset(ndv_b[:], -DELTA_V)
    # push-tail constants (f32, 32x32 world)
    ones32 = p_small.tile([32, 32], F32, tag="ones32")
    nc.vector.memset(ones32[:], 1.0)
    id32 = p_small.tile([32, 32], F32, tag="id32")
    nc.gpsimd.affine_select(
        id32[:], ones32[:], pattern=[[1, 32]], base=0,
        channel_multiplier=-1, compare_op=OP.is_equal, fill=0.0,
    )
    ones_k1 = p_small.tile([1, 32], F32, tag="ones_k1")
    nc.vector.memset(ones_k1[:], 1.0)
    ones32c = p_small.tile([32, 1], F32, tag="ones32c")
    nc.vector.memset(ones32c[:], 1.0)

    # ---------------- labels (bf16 from host) + counts (host bincount) ----
    nc.sync.dma_start(lab_b[:], lab_v)
    cnt_sb = p_small.tile([32, 1], F32, tag="cnt_sb")
    nc.sync.dma_start(cnt_sb[:], cnt_d[:].rearrange("(l o) -> l o", o=1))

    # one-hot rows (l-major): 32 TSP instrs, 4x DVE mode
    for l in range(NL):
        nc.vector.tensor_scalar(out=oh3[:, l, :], in0=lab_b[:],
                                scalar1=float(l + 1), scalar2=None,
                                op0=OP.is_equal)
    psum_a = ps_a.tile([32, D], F32, tag="psum_a")

    # ================= EPOCH 1: stream emb, phase A + block transpose =======
    # hib33: 32 chunks of [32 dims | 1 one] per buffer; the ones column rides
    # along in the phase-A matmul so counts land in psum_a[:, 32] for free.
    NBLK = npc // 32
    with tc.tile_pool(name="p_in", bufs=3) as p_in, \
         tc.tile_pool(name="p_hib", bufs=3) as p_hib:
        for b in range(NBLK):
            ta = p_in.tile([P, 1024], F32, tag="ta")
            nc.sync.dma_start(ta[:], emb_v[:, b * 1024:(b + 1) * 1024])
            hib = p_hib.tile([P, 1024], BF16, tag="hib")
            nc.scalar.copy(hib[:], ta[:])
            nc.vector.transpose(hi_st[:, b * 1024:(b + 1) * 1024], hib[:])
            for j in range(32):
                c = b * 32 + j
                nc.tensor.matmul(
                    psum_a[:], oh3[:, :, c],
                    hib[:, j * D:(j + 1) * D],
                    start=(c == 0), stop=(c == npc - 1),
                )

    # ================= means =================
    cnt_cl = p_small.tile([32, 1], F32, tag="cnt_cl")
    nc.vector.tensor_scalar(out=cnt_cl[:], in0=cnt_sb[:], scalar1=1.0,
                            scalar2=None, op0=OP.max)
    recip = p_small.tile([32, 1], F32, tag="recip")
    nc.vector.reciprocal(recip[:], cnt_cl[:])
    nrecip = p_small.tile([32, 1], F32, tag="nrecip")
    nc.vector.tensor_scalar(out=nrecip[:], in0=recip[:], scalar1=-1.0,
                            scalar2=None, op0=OP.mult)
    means_f = p_small.tile([32, 32], F32, tag="means_f")
    nc.vector.tensor_scalar(out=means_f[:], in0=psum_a[:], scalar1=recip[:],
                            scalar2=None, op0=OP.mult)
    negm_b = p_small.tile([32, 32], BF16, tag="negm_b")
    nc.scalar.activation(negm_b[:], psum_a[:], AF.Copy, scale=nrecip[:])
    # replicate into 4 diagonal blocks of negmT4 (partition-shifting DMAs)
    for q in range(4):
        nc.sync.dma_start(negmT4[SQ * q:SQ * (q + 1), SQ * q:SQ * q + 32],
                          negm_b[:])

    # ================= EPOCH 2: transposed phase B + phase C ================
    d2_bank = {}
    for ch in range(2):
        d2_bank[ch] = ps_d2.tile([P, H], F32, tag=f"d2_{ch}", name=f"d2_{ch}")
    labdr3 = lab16_d[:].rearrange("(q s c) -> q s c", q=4, s=SQ)
    for ch in range(2):
        for s in range(SQ):
            labT = p_t2.tile([P, H], BF16, tag="labT")
            nc.sync.dma_start(
                labT[:],
                labdr3[:, s, ch * H:(ch + 1) * H]
                .unsqueeze(1).broadcast_to([4, SQ, H]),
            )
            ohT = p_t2.tile([P, H], BF16, tag="ohT")
            nc.vector.tensor_scalar(out=ohT[:], in0=labT[:], scalar1=lvec[:],
                                    scalar2=None, op0=OP.is_equal)
            dpsum = ps_diff.tile([P, H], F32, tag="dpsum")
            nc.tensor.matmul(dpsum[:], negmT4[:], ohT[:], start=True, stop=False)
            nc.tensor.matmul(dpsum[:], id128[:], hst3[:, ch * H:(ch + 1) * H, s],
                             start=False, stop=True)
            sqd = p_t2.tile([P, H], BF16, tag="sqd")
            nc.scalar.activation(sqd[:], dpsum[:], AF.Square)
            nc.tensor.matmul(d2_bank[ch][:], ones_base[:, 31 - s:159 - s],
                             sqd[:], start=(s == 0), stop=(s == SQ - 1),
                             skip_group_check=True)
        dist = p_t2.tile([P, H], F32, tag="dist")
        nc.scalar.activation(dist[:], d2_bank[ch][:], AF.Sqrt, bias=eps_b[:])
        nc.scalar.activation(h_all[:, ch * H:(ch + 1) * H], dist[:],
                             AF.Relu, bias=ndv_b[:])
        for l in range(NL):
            prod = p_t2.tile([P, H], BF16, tag="prod")
            nc.vector.tensor_tensor(
                out=prod[:], in0=oh3[:, l, ch * H:(ch + 1) * H],
                in1=h_all[:, ch * H:(ch + 1) * H], op=OP.mult)
            col = segsum[:, ch * NL + l:ch * NL + l + 1]
            if l < 16:
                nc.scalar.activation(junk[:, 0:H], prod[:], AF.Copy,
                                     accum_out=col)
            else:
                nc.vector.reduce_sum(out=col, in_=prod[:], axis=AX.X)

    # phase C cross-partition reduce: two accumulating matmuls
    ps_seg = ps_misc.tile([32, 1], F32, tag="misc")
    nc.tensor.matmul(ps_seg[:], segsum[:, 0:NL], ones128f[:],
                     start=True, stop=False)
    nc.tensor.matmul(ps_seg[:], segsum[:, NL:2 * NL], ones128f[:],
                     start=False, stop=True)

    seg_mean = p_small.tile([32, 1], F32, tag="seg_mean")
    nc.vector.tensor_scalar(out=seg_mean[:], in0=ps_seg[:], scalar1=recip[:],
                            scalar2=None, op0=OP.mult)

    # ================= push tail (f32, tiny; same as baseline) =============
    mnsq = p_small.tile([32, 32], F32, tag="mnsq")
    nc.vector.tensor_tensor(out=mnsq[:], in0=means_f[:], in1=means_f[:], op=OP.mult)
    nrm2 = p_small.tile([32, 1], F32, tag="nrm2")
    nc.vector.reduce_sum(out=nrm2[:], in_=mnsq[:], axis=AX.X)
    nrm = p_small.tile([32, 1], F32, tag="nrm")
    nc.scalar.activation(nrm[:], nrm2[:], AF.Sqrt)
    nrm_cl = p_small.tile([32, 1], F32, tag="nrm_cl")
    nc.vector.tensor_scalar(out=nrm_cl[:], in0=nrm[:], scalar1=1e-12,
                            scalar2=None, op0=OP.max)
    rnrm = p_small.tile([32, 1], F32, tag="rnrm")
    nc.vector.reciprocal(rnrm[:], nrm_cl[:])
    mn = p_small.tile([32, 32], F32, tag="mn")
    nc.vector.tensor_scalar(out=mn[:], in0=means_f[:], scalar1=rnrm[:],
                            scalar2=None, op0=OP.mult)

    ps_mnt = ps_misc.tile([32, 32], F32, tag="misc")
    nc.tensor.transpose(ps_mnt[:], mn[:], id32[:])
    mnt = p_small.tile([32, 32], F32, tag="mnt")
    nc.vector.tensor_copy(mnt[:], ps_mnt[:])

    ps_g = ps_misc.tile([32, 32], F32, tag="misc")
    nc.tensor.matmul(ps_g[:], mnt[:], mnt[:], start=True, stop=True)

    mnsq2 = p_small.tile([32, 32], F32, tag="mnsq2")
    nc.vector.tensor_tensor(out=mnsq2[:], in0=mn[:], in1=mn[:], op=OP.mult)
    nsq = p_small.tile([32, 1], F32, tag="nsq")
    nc.vector.reduce_sum(out=nsq[:], in_=mnsq2[:], axis=AX.X)

    present = p_small.tile([32, 1], F32, tag="present")
    nc.vector.tensor_scalar(out=present[:], in0=cnt_sb[:], scalar1=0.0,
                            scalar2=None, op0=OP.is_gt)

    sq_a = p_small.tile([32, 32], F32, tag="sq_a")
    nc.vector.tensor_scalar(out=sq_a[:], in0=ps_g[:], scalar1=-2.0,
                            scalar2=nsq[:], op0=OP.mult, op1=OP.add)

    ps_row0 = ps_misc.tile([1, 32], F32, tag="misc")
    nc.tensor.matmul(ps_row0[:], nsq[:], id32[:], start=True, stop=True)
    nsqt_sb = p_small.tile([1, 32], F32, tag="nsqt_sb")
    nc.vector.tensor_copy(nsqt_sb[:], ps_row0[:])
    ps_row1 = ps_misc.tile([1, 32], F32, tag="misc")
    nc.tensor.matmul(ps_row1[:], present[:], id32[:], start=True, stop=True)
    prest_sb = p_small.tile([1, 32], F32, tag="prest_sb")
    nc.vector.tensor_copy(prest_sb[:], ps_row1[:])

    ps_bc = ps_misc.tile([32, 64], F32, tag="misc")
    nc.tensor.matmul(ps_bc[:, 0:32], ones_k1[:], nsqt_sb[:],
                     start=True, stop=True)
    nc.tensor.matmul(ps_bc[:, 32:64], ones_k1[:], prest_sb[:],
                     start=True, stop=True)
    nsq_j = p_small.tile([32, 32], F32, tag="nsq_j")
    nc.vector.tensor_copy(nsq_j[:], ps_bc[:, 0:32])
    pres_j = p_small.tile([32, 32], F32, tag="pres_j")
    nc.vector.tensor_copy(pres_j[:], ps_bc[:, 32:64])

    sq0 = p_small.tile([32, 32], F32, tag="sq0")
    nc.vector.tensor_tensor(out=sq0[:], in0=sq_a[:], in1=nsq_j[:], op=OP.add)
    sqc = p_small.tile([32, 32], F32, tag="sqc")
    nc.vector.tensor_scalar(out=sqc[:], in0=sq0[:], scalar1=0.0,
                            scalar2=None, op0=OP.max)
    dmat = p_small.tile([32, 32], F32, tag="dmat")
    nc.scalar.activation(dmat[:], sqc[:], AF.Sqrt, bias=eps_b[0:32, :])
    hp0 = p_small.tile([32, 32], F32, tag="hp0")
    nc.scalar.activation(hp0[:], dmat[:], AF.Relu, bias=ones32c[:], scale=-1.0)
    hp1 = p_small.tile([32, 32], F32, tag="hp1")
    nc.vector.tensor_scalar(out=hp1[:], in0=hp0[:], scalar1=present[:],
                            scalar2=None, op0=OP.mult)
    hp2 = p_small.tile([32, 32], F32, tag="hp2")
    nc.vector.tensor_tensor(out=hp2[:], in0=hp1[:], in1=pres_j[:], op=OP.mult)
    hp3 = p_small.tile([32, 32], F32, tag="hp3")
    nc.gpsimd.affine_select(hp3[:], hp2[:], pattern=[[1, 32]], base=0,
                            channel_multiplier=-1, compare_op=OP.is_gt, fill=0.0)
    pm1 = p_small.tile([32, 32], F32, tag="pm1")
    nc.vector.tensor_scalar(out=pm1[:], in0=pres_j[:], scalar1=present[:],
                            scalar2=None, op0=OP.mult)
    pm = p_small.tile([32, 32], F32, tag="pm")
    nc.gpsimd.affine_select(pm[:], pm1[:], pattern=[[1, 32]], base=0,
                            channel_multiplier=-1, compare_op=OP.is_gt, fill=0.0)
    hp_rs = p_small.tile([32, 1], F32, tag="hp_rs")
    nc.vector.reduce_sum(out=hp_rs[:], in_=hp3[:], axis=AX.X)
    pm_rs = p_small.tile([32, 1], F32, tag="pm_rs")
    nc.vector.reduce_sum(out=pm_rs[:], in_=pm[:], axis=AX.X)

    # ================= finals =================
    cat4 = p_small.tile([32, 4], F32, tag="cat4")
    nc.vector.tensor_copy(cat4[:, 0:1], seg_mean[:])
    nc.vector.tensor_copy(cat4[:, 1:2], present[:])
    nc.vector.tensor_copy(cat4[:, 2:3], hp_rs[:])
    nc.vector.tensor_copy(cat4[:, 3:4], pm_rs[:])
    ps_fin = ps_misc.tile([1, 4], F32, tag="misc")
    nc.tensor.matmul(ps_fin[:], ones32c[:], cat4[:], start=True, stop=True)
    sc = p_small.tile([1, 4], F32, tag="sc")
    nc.vector.tensor_copy(sc[:], ps_fin[:])

    res_sb = p_small.tile([1, 8], F32, tag="res_sb")
    nc.vector.memset(res_sb[:], 0.0)
    t1 = p_small.tile([1, 1], F32, tag="t1")
    nc.vector.tensor_scalar(out=t1[:], in0=sc[:, 1:2], scalar1=1e-6,
                            scalar2=None, op0=OP.add)
    r1 = p_small.tile([1, 1], F32, tag="r1")
    nc.vector.reciprocal(r1[:], t1[:])
    nc.vector.tensor_tensor(out=res_sb[:, 0:1], in0=sc[:, 0:1], in1=r1[:],
                            op=OP.mult)
    t2 = p_small.tile([1, 1], F32, tag="t2")
    nc.vector.tensor_scalar(out=t2[:], in0=sc[:, 3:4], scalar1=1e-6,
                            scalar2=None, op0=OP.add)
    r2 = p_small.tile([1, 1], F32, tag="r2")
    nc.vector.reciprocal(r2[:], t2[:])
    pb0 = p_small.tile([1, 1], F32, tag="pb0")
    nc.vector.tensor_tensor(out=pb0[:], in0=sc[:, 2:3], in1=r2[:], op=OP.mult)
    gate = p_small.tile([1, 1], F32, tag="gate")
    nc.vector.tensor_scalar(out=gate[:], in0=sc[:, 1:2], scalar1=1.0,
                            scalar2=None, op0=OP.is_gt)
    nc.vector.tensor_tensor(out=res_sb[:, 1:2], in0=pb0[:], in1=gate[:],
                            op=OP.mult)

    nc.sync.dma_start(res_d[:], res_sb[:])


def build_program(npc):
    n = npc * 128
    nc = bacc.Bacc("TRN2", target_bir_lowering=False, debug=False)
    emb_d = nc.dram_tensor("emb", [n, D], F32, kind="ExternalInput")
    lab16_d = nc.dram_tensor("lab16", [n], BF16, kind="ExternalInput")
    cnt_d = nc.dram_tensor("cnt", [32], F32, kind="ExternalInput")
    res_d = nc.dram_tensor("res", [1, 8], F32, kind="ExternalOutput")
    with tile.TileContext(nc) as tc:
        with ExitStack() as ctx:
            tc.ctx = ctx
            emit(tc, emb_d, lab16_d, cnt_d, res_d, npc)
    nc.compile()
    return nc


_NC_CACHE = {}


def _get_nc(npc):
    if npc not in _NC_CACHE:
        _NC_CACHE[npc] = build_program(npc)
    return _NC_CACHE[npc]


def kernel(embeddings, labels):
    embeddings = np.asarray(embeddings, dtype=np.float32)
    labels = np.asarray(labels, dtype=np.int32)
    bsz = embeddings.shape[0]
    npc = embeddings.shape[1] // 128
    nc = _get_nc(npc)

    from concourse.bass_utils import run_bass_kernel_spmd

    import ml_dtypes
    lab16 = labels.astype(np.float32).astype(ml_dtypes.bfloat16)
    counts = np.stack([
        np.bincount(labels[b], minlength=33)[1:33].astype(np.float32)
        for b in range(bsz)
    ])
    in_maps = [
        {"emb": np.ascontiguousarray(embeddings[b]),
         "lab16": np.ascontiguousarray(lab16[b]),
         "cnt": counts[b]}
        for b in range(bsz)
    ]
    out = run_bass_kernel_spmd(nc, in_maps, list(range(bsz)))
    res = np.stack([out.results[b]["res"][0] for b in range(bsz)])
    pull = res[:, 0].sum() / bsz
    push = res[:, 1].sum() / bsz
    return np.stack([pull + push, pull, push]).astype(np.float32)
